# revision 1
# baseline (speedup 1.0000x reference)
"""Trainium2 Bass kernel for a causal self-attention block (nanogpt-style).

Full inputs -> full output. Internally: 16 heads sharded as 2 heads/core
across 8 NeuronCores (tensor-parallel); each core computes its heads'
QKV projection, per-head RMSNorm + RoPE, causal flash-style attention
(no-max softmax: scores are bounded since q,k are RMS-normalized), and a
partial c_proj over its 128-dim slice of the residual. The host sums the
8 partial outputs.

Layout strategy per core (device):
  - QKV matmul in natural [t, e] layout (lhsT = xT tiles, rhs = W^T slice)
  - norm + rope on the vector engine in natural layout (free-dim reductions)
  - PE-transpose q,k -> packed QT/KT [128(2h x 64d), T]
  - attention: S^T = K_chunk^T.T @ Q^T per (qblock 512, tchunk 128) with
    2-head row-tiling of the PE array; exp on the scalar engine
    (scale=0.12 folded in); y^T accumulated in PSUM via lhsT=[V|1]
    (ones column gives the softmax denominator for free)
  - c_proj: lhsT = y^T tiles, rhs = Wc^T slice -> partial out [T, 1024]
"""

import numpy as np

DIM = 1024
NH = 16
HD = 64
SCALE = 0.12
NC_CORES = 8
HPC = NH // NC_CORES  # 2 heads per core


def _build(T=4096):
    import concourse.bass as bass
    import concourse.tile as tile
    from concourse import mybir

    f32 = mybir.dt.float32
    f32r = mybir.dt.float32r
    AF = mybir.ActivationFunctionType

    NTT = T // 128   # 128-row t tiles
    NQB = T // 512   # 512-col q blocks
    NG = max(1, NTT // 8)     # norm/rope groups of 8 t-tiles
    GT = NTT // NG            # t-tiles per group
    EPS = float(np.finfo(np.float32).eps)

    import os
    variant = os.environ.get('KVARIANT', 'all')
    nc = bass.Bass("TRN2", target_bir_lowering=False, debug=False,
                   num_devices=NC_CORES)

    xT = nc.declare_dram_parameter("xT", [DIM, T], f32, isOutput=False).ap()
    wT = nc.declare_dram_parameter("wT", [DIM, 384], f32, isOutput=False).ap()
    ve = nc.declare_dram_parameter("ve", [T, 192], f32, isOutput=False).ap()
    one1 = nc.declare_dram_parameter("one1", [1, 64], f32, isOutput=False).ap()
    onesd = nc.declare_dram_parameter("onesd", [128, 32], f32, isOutput=False).ap()
    cosn = nc.declare_dram_parameter("cosn", [T, 16], f32, isOutput=False).ap()
    sinn = nc.declare_dram_parameter("sinn", [T, 16], f32, isOutput=False).ap()
    tri = nc.declare_dram_parameter("tri", [128, 128], f32, isOutput=False).ap()
    iden = nc.declare_dram_parameter("iden", [128, 128], f32, isOutput=False).ap()
    wcT = nc.declare_dram_parameter("wcT", [128, DIM], f32, isOutput=False).ap()
    outp = nc.declare_dram_parameter("outp", [T, DIM], f32, isOutput=True).ap()

    def r(ap):
        return ap.bitcast(f32r)

    with tile.TileContext(nc) as tc:
        with (
            tc.tile_pool(name="consts", bufs=1) as consts,
            tc.tile_pool(name="persist", bufs=1) as persist,
            tc.tile_pool(name="xstream", bufs=3) as xstream,
            tc.tile_pool(name="tmp", bufs=3) as tmp,
            tc.tile_pool(name="pt", bufs=4) as ptpool,
            tc.tile_pool(name="small", bufs=4) as small,
            tc.tile_pool(name="scratch", bufs=4, space="PSUM") as psc,
            tc.tile_pool(name="ybank", bufs=4, space="PSUM") as pyb,
        ):
            # ---- constants ----
            tri_sb = consts.tile([128, 128], f32, tag="tri")
            nc.sync.dma_start(tri_sb[:, :], tri)
            id_sb = consts.tile([128, 128], f32, tag="iden")
            nc.sync.dma_start(id_sb[:, :], iden)
            cos_sb = consts.tile([128, NTT, 16], f32, tag="cos")
            nc.sync.dma_start(cos_sb[:, :, :],
                              cosn.rearrange("(tt p) i -> p tt i", p=128))
            sin_sb = consts.tile([128, NTT, 16], f32, tag="sin")
            nc.sync.dma_start(sin_sb[:, :, :],
                              sinn.rearrange("(tt p) i -> p tt i", p=128))
            w_sb = consts.tile([128, 8, 384], f32r, tag="w")
            nc.sync.dma_start(w_sb[:, :, :],
                              wT.bitcast(f32r).rearrange("(dc p) e -> p dc e", p=128))
            wc_sb = consts.tile([128, DIM], f32r, tag="wc")
            nc.sync.dma_start(wc_sb[:, :], wcT.bitcast(f32r))
            eps_sb = consts.tile([128, 1], f32, tag="eps")
            nc.gpsimd.memset(eps_sb[:, :], EPS)
            ones1 = consts.tile([1, 64], f32r, tag="ones1")
            nc.sync.dma_start(ones1[:, :], one1.bitcast(f32r))


            q_nat = persist.tile([128, NTT, 128], f32, tag="qnat")
            k_nat = persist.tile([128, NTT, 128], f32, tag="knat")
            v_sb = persist.tile([128, NTT, 192], f32r, tag="v")
            QT = persist.tile([128, T], f32r, tag="QT")
            KT = persist.tile([128, T], f32r, tag="KT")
            yT = persist.tile([128, T], f32r, tag="yT")

            # v value-embedding slices (lambda-scaled on host), with the
            # softmax-denominator ones columns baked in at cols 64 and 129
            nc.sync.dma_start(v_sb[:, :, :],
                              ve.bitcast(f32r).rearrange("(tt p) d -> p tt d", p=128))

            # ---- phase 1: qkv projection, natural layout ----
            for tt in range(NTT):
                xt = xstream.tile([128, 8, 128], f32r, tag="xt")
                nc.sync.dma_start(
                    xt[:, :, :],
                    xT.bitcast(f32r)[:, 128 * tt:128 * tt + 128]
                    .rearrange("(dc p) t -> p dc t", p=128))
                ps = psc.tile([128, 384], f32, tag="ps1")
                for dc in range(8):
                    nc.tensor.matmul(ps[:, :], r(xt[:, dc, :]), r(w_sb[:, dc, :]),
                                     start=(dc == 0), stop=(dc == 7))
                nc.vector.tensor_copy(q_nat[:, tt, :], ps[:, 0:128])
                nc.vector.tensor_copy(k_nat[:, tt, :], ps[:, 128:256])
                nc.vector.tensor_add(v_sb[:, tt, 0:64], ps[:, 256:320],
                                     v_sb[:, tt, 0:64])
                nc.vector.tensor_add(v_sb[:, tt, 96:160], ps[:, 320:384],
                                     v_sb[:, tt, 96:160])

            # ---- phase 2: rms norm + rope (natural layout, free-dim ops) ----
            for g in range(NG):
                gsl = slice(GT * g, GT * g + GT)
                for nat in (q_nat, k_nat):
                    xg = nat[:, gsl, :]                       # [128, GT, 128]
                    xg4 = nat[:, gsl, :].rearrange("p a (h d) -> p a h d", h=2)
                    sq = tmp.tile([128, GT * 128], f32, tag="sq")
                    nc.vector.tensor_mul(sq[:, :], xg, xg)
                    ssum = small.tile([128, GT, 2], f32, tag="ssum")
                    nc.vector.reduce_sum(
                        ssum[:, :, :],
                        sq[:, :].rearrange("p (a h d) -> p a h d", a=GT, h=2),
                        axis=mybir.AxisListType.X)
                    sstd = small.tile([128, GT, 2], f32, tag="sstd")
                    nc.scalar.activation(sstd[:, :, :], ssum[:, :, :],
                                         AF.Sqrt, bias=eps_sb[:, :],
                                         scale=1.0 / HD)
                    rinv = small.tile([128, GT, 2], f32, tag="rinv")
                    nc.vector.reciprocal(rinv[:, :, :], sstd[:, :, :])
                    nc.vector.tensor_mul(
                        xg4, xg4,
                        rinv[:, :, :].broadcast_to((128, GT, 2, HD)))
                    # rope on pairs (d, d+32), d in [0,16)
                    x1 = nat[:, gsl, :].rearrange("p a (h d) -> p a h d", h=2)[:, :, :, 0:16]
                    x2 = nat[:, gsl, :].rearrange("p a (h d) -> p a h d", h=2)[:, :, :, 32:48]
                    cg = (cos_sb[:, gsl, :].rearrange("p a i -> p a () i")
                          .broadcast_to((128, GT, 2, 16)))
                    sg = (sin_sb[:, gsl, :].rearrange("p a i -> p a () i")
                          .broadcast_to((128, GT, 2, 16)))
                    t1 = tmp.tile([128, GT, 2, 16], f32, tag="t1")
                    t2 = tmp.tile([128, GT, 2, 16], f32, tag="t2")
                    t3 = tmp.tile([128, GT, 2, 16], f32, tag="t3")
                    t4 = tmp.tile([128, GT, 2, 16], f32, tag="t4")
                    nc.vector.tensor_mul(t1[:, :, :, :], x1, cg)
                    nc.vector.tensor_mul(t2[:, :, :, :], x2, sg)
                    nc.vector.tensor_mul(t3[:, :, :, :], x1, sg)
                    nc.vector.tensor_mul(t4[:, :, :, :], x2, cg)
                    nc.vector.tensor_add(x1, t1[:, :, :, :], t2[:, :, :, :])
                    nc.vector.tensor_sub(x2, t4[:, :, :, :], t3[:, :, :, :])

            # ---- phase 3: transpose q,k -> QT/KT [128(2h*64d), T] ----
            for tt in range(NTT):
                for nat, dstT in ((q_nat, QT), (k_nat, KT)):
                    pt = psc.tile([128, 128], f32, tag="ps1")
                    nc.tensor.transpose(pt[:, :], nat[:, tt, :], id_sb[:, :])
                    nc.vector.tensor_copy(dstT[:, 128 * tt:128 * tt + 128],
                                          pt[:, :])

            if variant == 'p3':
                # bisect: stop after transposes, dump QT slice
                ob0 = ptpool.tile([128, 512], f32, tag="pt", name="obp3")
                nc.vector.tensor_copy(ob0[:, :], QT[:, 0:512].bitcast(f32))
                nc.sync.dma_start(outp[0:128, 0:512], ob0[:, :])

            # ---- phase 4: causal attention per q-block of 512 ----
            for qb in range(NQB if variant != 'p3' else 0):
                q0 = 512 * qb
                ntc = 4 * (qb + 1)
                py = [pyb.tile([96, 512], f32, tag="py", name=f"py{qb}_{h}")
                      for h in range(2)]
                stage = []  # software pipeline: delay y-matmul by one tchunk
                for tc_ in range(ntc):
                    p = tc_ - 4 * qb
                    ql = max(0, 128 * p)
                    ts = slice(128 * tc_, 128 * tc_ + 128)
                    pts = []
                    for h in range(1 if variant == 'h0only' else 2):
                        hp = slice(64 * h, 64 * h + 64)
                        s = psc.tile([128, 512], f32, tag="ps1",
                                     name=f"s{qb}_{tc_}_{h}")
                        nc.tensor.matmul(
                            s[:, ql:512], r(KT[hp, ts]),
                            r(QT[hp, q0 + ql:q0 + 512]),
                            start=True, stop=True,
                            tile_position=(64 * h, 0))
                        pt = ptpool.tile([128, 512], f32r, tag="pt",
                                         name=f"ptile{qb}_{tc_}_{h}")
                        nc.scalar.activation(pt[:, ql:512], s[:, ql:512],
                                             AF.Exp, scale=SCALE)
                        if p >= 0:
                            nc.vector.tensor_mul(pt[:, ql:ql + 128],
                                                 pt[:, ql:ql + 128],
                                                 tri_sb[:, :])
                        pts.append(pt)
                    stage.append((tc_, ql, pts))
                    if variant in ('sonly', 'h0only'):
                        stage.pop(0)
                        continue
                    if len(stage) == 2:
                        _emit_y(nc, r, v_sb, py, stage.pop(0), ntc, variant)
                while stage:
                    _emit_y(nc, r, v_sb, py, stage.pop(0), ntc, variant)
                # normalize by the ones-column denominator; write yT
                for h in range(1 if variant == 'h0only' else 2):
                    if variant in ('sonly', 'h0only'):
                        nc.vector.tensor_copy(
                            yT[64 * h:64 * h + 64, q0:q0 + 512],
                            pts[h][64 * h:64 * h + 64, 0:512])
                        continue
                    if variant in ('nonorm', 'ynoones', 'y63'):
                        nc.vector.tensor_copy(
                            yT[64 * h:64 * h + 64, q0:q0 + 512],
                            py[h][0:64, :])
                        continue
                    rden = small.tile([1, 512], f32r, tag="rden")
                    with nc.allow_low_precision(reason="f32r recip for PE"):
                        nc.vector.reciprocal(rden[:, :], py[h][64:65, :])
                    rrep = psc.tile([64, 512], f32, tag="ps1",
                                    name=f"rrep{qb}_{h}")
                    nc.tensor.matmul(rrep[:, :], r(ones1[:, :]),
                                     r(rden[:, :]), start=True, stop=True)
                    rrs = tmp.tile([64, 512], f32, tag="rrs",
                                   name=f"rrs{qb}_{h}")
                    nc.vector.tensor_copy(rrs[:, :], rrep[:, :])
                    nc.vector.tensor_mul(yT[64 * h:64 * h + 64, q0:q0 + 512],
                                         py[h][0:64, :], rrs[:, :])
                # ---- phase 5: c_proj for the 4 finished t-tiles ----
                for tt in range(4 * qb, 4 * qb + 4):
                    ts2 = slice(128 * tt, 128 * tt + 128)
                    for half in range(2):
                        po = pyb.tile([128, 512], f32, tag="py",
                                      name=f"po{tt}_{half}")
                        nc.tensor.matmul(po[:, :], r(yT[:, ts2]),
                                         r(wc_sb[:, 512 * half:512 * half + 512]),
                                         start=True, stop=True)
                        ob = ptpool.tile([128, 512], f32, tag="pt",
                                         name=f"ob{tt}_{half}")
                        nc.vector.tensor_copy(ob[:, :], po[:, :])
                        nc.sync.dma_start(outp[ts2, 512 * half:512 * half + 512],
                                          ob[:, :])
    _cap_matmul_waits(nc)
    return nc


def _cap_matmul_waits(nc, limit=1):
    """walrus supports few (often one) sync-wait slots per lowered
    instruction; move excess waits onto same-engine nops inserted just
    before, so the sequencer blocks identically but each instruction
    carries at most `limit` waits."""
    import bass_rust
    from concourse import mybir

    eng = {
        mybir.EngineType.PE: nc.tensor,
        mybir.EngineType.DVE: nc.vector,
        mybir.EngineType.Activation: nc.scalar,
        mybir.EngineType.Pool: nc.gpsimd,
        mybir.EngineType.SP: nc.sync,
    }

    def make_nop(e):
        eng[e].nop()
        fn = nc.m.functions[0]
        for obb in fn.blocks:
            if (obb.instructions
                    and type(obb.instructions[-1]).__name__ == 'InstNoOp'):
                return obb.instructions.pop()
        raise AssertionError('nop not found')

    fn = nc.m.functions[0]
    for bb in fn.blocks:
        il = bb.instructions
        i = 0
        while i < len(il):
            inst = il[i]
            si = inst.sync_info
            if (si and si.on_wait and len(si.on_wait) > limit
                    and inst.engine in eng
                    and type(inst).__name__ != 'InstNoOp'):
                waits = list(si.on_wait)
                keep, excess = waits[-limit:], waits[:-limit]
                for w in excess:
                    nop = make_nop(inst.engine)
                    nop.sync_info = bass_rust.SyncInfo(on_wait=[w],
                                                       on_update=[])
                    il.insert(i, nop)
                    i += 1
                inst.sync_info = bass_rust.SyncInfo(
                    on_wait=keep, on_update=list(si.on_update))
            i += 1


def _emit_y(nc, r, v_sb, py, item, ntc, variant='all', ones32=None):
    # y accumulation with lhsT = [V_h(64) | ones(32)]: M=96 (f32r fused
    # matmuls fault HW for M=65 and reject nonzero dst partition bases,
    # so the denominator rides along as 32 redundant ones columns).
    tc_, ql, pts = item
    last = tc_ == ntc - 1
    for h in range(2):
        nc.tensor.matmul(py[h][0:96, ql:512],
                         r(v_sb[:, tc_, 96 * h:96 * h + 96]),
                         r(pts[h][:, ql:512]),
                         start=(tc_ == 0), stop=last)


def _host_prep(x, ve, qkv_w, lambdas, c_proj_w, T):
    xT = np.ascontiguousarray(x.reshape(T, DIM).T)
    af = (1.0 / 1024.0) ** np.linspace(0.0, 1.0, HD // 4, dtype=np.float32)
    theta = np.arange(T, dtype=np.float32)[:, None] * af[None, :]
    cos = np.ascontiguousarray(np.cos(theta), np.float32)
    sin = np.ascontiguousarray(np.sin(theta), np.float32)
    tri = np.ascontiguousarray(np.triu(np.ones((128, 128), np.float32)))
    iden = np.ascontiguousarray(np.eye(128, dtype=np.float32))
    lam = np.asarray(lambdas, np.float32)
    vef = ve.reshape(T, DIM)
    in_maps = []
    for c in range(NC_CORES):
        sl = slice(128 * c, 128 * c + 128)
        wq = qkv_w[0][sl]
        wk = qkv_w[1][sl]
        wv = qkv_w[2][sl] * lam[0]
        wTl = np.ascontiguousarray(np.concatenate([wq, wk, wv], 0).T)
        ve_l = np.ones((T, 192), np.float32)
        ve_l[:, 0:64] = vef[:, sl.start:sl.start + 64] * lam[1]
        ve_l[:, 96:160] = vef[:, sl.start + 64:sl.stop] * lam[1]
        wcTl = np.ascontiguousarray(c_proj_w[:, sl].T)
        in_maps.append(dict(xT=xT, wT=wTl, ve=ve_l, cosn=cos, sinn=sin,
                            tri=tri, iden=iden, wcT=wcTl,
                            one1=np.ones((1, 64), np.float32),
                            onesd=np.ones((128, 32), np.float32)))
    return in_maps


LAST_RESULTS = None


def kernel(x, ve, qkv_w, lambdas, c_proj_w):
    import sys
    if '/opt/trn_rl_repo' not in sys.path:
        sys.path.insert(0, '/opt/trn_rl_repo')
    from concourse.bass_utils import run_bass_kernel_spmd

    x = np.asarray(x, np.float32)
    T = x.shape[1]
    in_maps = _host_prep(np.asarray(x, np.float32), np.asarray(ve, np.float32),
                         np.asarray(qkv_w, np.float32),
                         np.asarray(lambdas, np.float32),
                         np.asarray(c_proj_w, np.float32), T)
    nc = _build(T)
    res = run_bass_kernel_spmd(nc, in_maps, core_ids=list(range(NC_CORES)))
    global LAST_RESULTS
    LAST_RESULTS = res
    out = np.zeros((T, DIM), np.float32)
    for rmap in res.results:
        out += rmap["outp"]
    return out.reshape(1, T, DIM)



# revision 31
# speedup vs baseline: 1.9197x; 1.9197x over previous
"""Trainium2 Bass kernel for a causal self-attention block (nanogpt-style).

Full inputs -> full output. 16 heads sharded 2/core across 8 NeuronCores
(tensor-parallel); each core computes its heads' QKV projection, per-head
RMSNorm + RoPE, causal no-max-softmax attention (scores bounded: q,k are
RMS-normalized so |s|<=64), and a partial c_proj over its 128-dim slice of
the residual. Host sums the 8 bf16 partial outputs in f32.

v2 (bf16 rewrite of the f32r baseline, 650us -> target <250us):
  - all matmuls in bf16: 1 cyc/row at any N (f32r needs N>=256 and measured
    ~2x slower on HW), FWL fast weight loads, and half the DMA/SBUF traffic
  - exp fused across both heads per chunk: one ACTIVATE over a [128,2,N]
    view of a 2-bank PSUM score tile (fewer, larger Act instructions)
  - y accumulation col-packed: both heads' PV matmuls (M=64 each) run
    concurrently in one PSUM bank via tile_position=(0,0)/(0,64); softmax
    denominators come from 4 concurrent M=32 ones-matmul strips
  - softmax normalize: reciprocal_approx_fast on [2,512] rows (the
    baseline's [1,512] nc.vector.reciprocal cost 55us total), PE-replicated
    across partitions, folded into the py->yT eviction multiply
"""

import numpy as np

DIM = 1024
NH = 16
HD = 64
SCALE = 0.12
NC_CORES = 8
HPC = NH // NC_CORES  # 2 heads per core


def _build(T=4096):
    import concourse.bass as bass
    import concourse.tile as tile
    from concourse import mybir

    f32 = mybir.dt.float32
    f32r = mybir.dt.float32r
    bf16 = mybir.dt.bfloat16
    AF = mybir.ActivationFunctionType

    NTT = T // 128   # 32 t-tiles of 128
    NQB = T // 512   # 8 q-blocks of 512
    NG = max(1, NTT // 8)     # norm/rope groups of 8 t-tiles
    GT = NTT // NG
    EPS = float(np.finfo(np.float32).eps)

    nc = bass.Bass("TRN2", target_bir_lowering=False, debug=False,
                   num_devices=NC_CORES)

    xT = nc.declare_dram_parameter("xT", [DIM, T], bf16, isOutput=False).ap()
    wT = nc.declare_dram_parameter("wT", [DIM, 384], bf16, isOutput=False).ap()
    ve = nc.declare_dram_parameter("ve", [T, 192], bf16, isOutput=False).ap()
    sel = nc.declare_dram_parameter("sel", [64, 128], bf16, isOutput=False).ap()
    cosn = nc.declare_dram_parameter("cosn", [T, 16], bf16, isOutput=False).ap()
    sinn = nc.declare_dram_parameter("sinn", [T, 16], bf16, isOutput=False).ap()
    tri = nc.declare_dram_parameter("tri", [128, 128], bf16, isOutput=False).ap()
    iden = nc.declare_dram_parameter("iden", [128, 128], bf16, isOutput=False).ap()
    wcT = nc.declare_dram_parameter("wcT", [128, DIM], bf16, isOutput=False).ap()
    outp = nc.declare_dram_parameter("outp", [T, DIM], bf16, isOutput=True).ap()
    import os
    debug = os.environ.get('KDEBUG', '0') == '1'
    if debug:
        dQT = nc.declare_dram_parameter("dQT", [128, T], bf16, isOutput=True).ap()
        dKT = nc.declare_dram_parameter("dKT", [128, T], bf16, isOutput=True).ap()
        dV = nc.declare_dram_parameter("dV", [128, NTT * 192], bf16, isOutput=True).ap()
        dYT = nc.declare_dram_parameter("dYT", [128, T], bf16, isOutput=True).ap()
        dDEN = nc.declare_dram_parameter("dDEN", [64, T], bf16, isOutput=True).ap()
        dPT = nc.declare_dram_parameter("dPT", [128, 1024], bf16, isOutput=True).ap()

    def r(ap):
        return ap.bitcast(f32r)

    with tile.TileContext(nc) as tc:
        with (
            tc.tile_pool(name="consts", bufs=1) as consts,
            tc.tile_pool(name="persist", bufs=1) as persist,
            tc.tile_pool(name="xstream", bufs=3) as xstream,
            tc.tile_pool(name="tmp", bufs=3) as tmp,
            tc.tile_pool(name="pt", bufs=3) as ptpool,
            tc.tile_pool(name="ob", bufs=3) as obpool,
            tc.tile_pool(name="small", bufs=4) as small,
            tc.tile_pool(name="sc", bufs=2, space="PSUM") as psc,      # 2x2 banks
            tc.tile_pool(name="py", bufs=2, space="PSUM") as pyp,      # 2 banks
            tc.tile_pool(name="dn", bufs=1, space="PSUM") as pdn,      # 1 bank
            tc.tile_pool(name="po", bufs=1, space="PSUM") as ppo,      # 1 bank
        ):
            # ---- constants ----
            tri_sb = consts.tile([128, 128], bf16, tag="tri")
            nc.sync.dma_start(tri_sb[:, :], tri)
            id_sb = consts.tile([128, 128], bf16, tag="iden")
            nc.sync.dma_start(id_sb[:, :], iden)
            cos_sb = consts.tile([128, NTT, 16], bf16, tag="cos")
            nc.sync.dma_start(cos_sb[:, :, :],
                              cosn.rearrange("(tt p) i -> p tt i", p=128))
            sin_sb = consts.tile([128, NTT, 16], bf16, tag="sin")
            nc.sync.dma_start(sin_sb[:, :, :],
                              sinn.rearrange("(tt p) i -> p tt i", p=128))
            w_sb = consts.tile([128, 8, 384], bf16, tag="w")
            nc.sync.dma_start(w_sb[:, :, :],
                              wT.rearrange("(dc p) e -> p dc e", p=128))
            wc_sb = consts.tile([128, DIM], bf16, tag="wc")
            nc.sync.dma_start(wc_sb[:, :], wcT)
            eps_sb = consts.tile([128, 1], f32, tag="eps")
            nc.gpsimd.memset(eps_sb[:, :], EPS)
            sel_sb = consts.tile([64, 128], bf16, tag="sel")
            nc.sync.dma_start(sel_sb[:, :], sel)

            # den staging (rows {0,32} hold h0,h1; rest stays 1.0 so a single
            # [33,512] reciprocal per q-block is safe)
            dtot = persist.tile([64, 512], f32, tag="dtot")
            nc.gpsimd.memset(dtot[:, :], 1.0)
            rden = persist.tile([64, 512], f32, tag="rden")
            rdenb = persist.tile([64, 512], bf16, tag="rdenb")

            q_nat = persist.tile([128, NTT, 128], bf16, tag="qnat")
            k_nat = persist.tile([128, NTT, 128], bf16, tag="knat")
            # rider layout: v0 | ones | v1 | ones (denominator rides as
            # redundant ones columns in the M=96 PV matmul)
            v_sb = persist.tile([128, NTT, 192], bf16, tag="v")
            QT = persist.tile([128, T], bf16, tag="QT")
            KT = persist.tile([128, T], bf16, tag="KT")
            yT = persist.tile([128, T], bf16, tag="yT")

            # v preloaded with lambda1 * ve slice (host-scaled)
            nc.sync.dma_start(v_sb[:, :, :],
                              ve.rearrange("(tt p) d -> p tt d", p=128))

            # ---- phase 1: qkv projection, natural layout ----
            for tt in range(NTT):
                xt = xstream.tile([128, 8, 128], bf16, tag="xt")
                nc.sync.dma_start(
                    xt[:, :, :],
                    xT[:, 128 * tt:128 * tt + 128]
                    .rearrange("(dc p) t -> p dc t", p=128))
                ps = psc.tile([128, 1024], f32, tag="sc", name=f"ps1_{tt}")
                for dc in range(8):
                    nc.tensor.matmul(ps[:, 0:384], xt[:, dc, :], w_sb[:, dc, :],
                                     start=(dc == 0), stop=(dc == 7))
                nc.vector.tensor_copy(q_nat[:, tt, :], ps[:, 0:128])
                nc.vector.tensor_copy(k_nat[:, tt, :], ps[:, 128:256])
                nc.vector.tensor_add(v_sb[:, tt, 0:64], ps[:, 256:320],
                                     v_sb[:, tt, 0:64])
                nc.vector.tensor_add(v_sb[:, tt, 96:160], ps[:, 320:384],
                                     v_sb[:, tt, 96:160])

            # ---- phase 2: rms norm + rope (natural layout, free-dim ops) ----
            for g in range(NG):
                gsl = slice(GT * g, GT * g + GT)
                for nat in (q_nat, k_nat):
                    xg = nat[:, gsl, :]                       # [128, GT, 128]
                    xg4 = nat[:, gsl, :].rearrange("p a (h d) -> p a h d", h=2)
                    sq = tmp.tile([128, GT * 128], bf16, tag="sq")
                    nc.vector.tensor_mul(sq[:, :], xg, xg)
                    ssum = small.tile([128, GT, 2], f32, tag="ssum")
                    nc.vector.reduce_sum(
                        ssum[:, :, :],
                        sq[:, :].rearrange("p (a h d) -> p a h d", a=GT, h=2),
                        axis=mybir.AxisListType.X)
                    sstd = small.tile([128, GT, 2], f32, tag="sstd")
                    nc.scalar.activation(sstd[:, :, :], ssum[:, :, :],
                                         AF.Sqrt, bias=eps_sb[:, :],
                                         scale=1.0 / HD)
                    rinv = small.tile([128, GT, 2], f32, tag="rinv")
                    nc.vector.reciprocal(rinv[:, :, :], sstd[:, :, :])
                    rinvb = small.tile([128, GT, 2], bf16, tag="rinvb")
                    nc.vector.tensor_copy(rinvb[:, :, :], rinv[:, :, :])
                    nc.vector.tensor_mul(
                        xg4, xg4,
                        rinvb[:, :, :].broadcast_to((128, GT, 2, HD)))
                    # rope on pairs (d, d+32), d in [0,16)
                    x1 = nat[:, gsl, :].rearrange("p a (h d) -> p a h d", h=2)[:, :, :, 0:16]
                    x2 = nat[:, gsl, :].rearrange("p a (h d) -> p a h d", h=2)[:, :, :, 32:48]
                    cg = (cos_sb[:, gsl, :].rearrange("p a i -> p a () i")
                          .broadcast_to((128, GT, 2, 16)))
                    sg = (sin_sb[:, gsl, :].rearrange("p a i -> p a () i")
                          .broadcast_to((128, GT, 2, 16)))
                    t1 = tmp.tile([128, GT, 2, 16], bf16, tag="t1")
                    t2 = tmp.tile([128, GT, 2, 16], bf16, tag="t2")
                    t3 = tmp.tile([128, GT, 2, 16], bf16, tag="t3")
                    t4 = tmp.tile([128, GT, 2, 16], bf16, tag="t4")
                    nc.vector.tensor_mul(t1[:, :, :, :], x1, cg)
                    nc.vector.tensor_mul(t2[:, :, :, :], x2, sg)
                    nc.vector.tensor_mul(t3[:, :, :, :], x1, sg)
                    nc.vector.tensor_mul(t4[:, :, :, :], x2, cg)
                    nc.vector.tensor_add(x1, t1[:, :, :, :], t2[:, :, :, :])
                    nc.vector.tensor_sub(x2, t4[:, :, :, :], t3[:, :, :, :])

            # ---- phase 3: transpose q,k -> QT/KT [128(2h*64d), T] ----
            for tt in range(NTT):
                for nat, dstT in ((q_nat, QT), (k_nat, KT)):
                    pt_ps = ppo.tile([128, 128], bf16, tag="po",
                                     name=f"tp{tt}")
                    nc.tensor.transpose(pt_ps[:, :], nat[:, tt, :],
                                        id_sb[:, :])
                    nc.vector.tensor_copy(dstT[:, 128 * tt:128 * tt + 128],
                                          pt_ps[:, :])

            # ---- phase 4: causal attention per q-block of 512 ----
            for qb in range(NQB):
                q0 = 512 * qb
                ntc = 4 * (qb + 1)
                py = [pyp.tile([128, 512], f32, tag="py", name=f"py{qb}_{h}")
                      for h in range(2)]
                stage = []  # software pipeline: y lags scores by one pair
                for pi in range(ntc // 2):
                    pts = []
                    for c in (2 * pi, 2 * pi + 1):
                        p = c - 4 * qb
                        ql = max(0, 128 * p)
                        ts = slice(128 * c, 128 * c + 128)
                        s = psc.tile([128, 1024], f32, tag="sc",
                                     name=f"s{qb}_{c}")
                        for h in range(2):
                            hp = slice(64 * h, 64 * h + 64)
                            nc.tensor.matmul(
                                s[:, 512 * h + ql:512 * h + 512],
                                KT[hp, ts], QT[hp, q0 + ql:q0 + 512],
                                start=True, stop=True,
                                tile_position=(64 * h, 0))
                        pt = ptpool.tile([128, 2, 512], bf16, tag="pt",
                                         name=f"ptile{qb}_{c}")
                        nc.scalar.activation(
                            pt[:, :, ql:512],
                            s[:, :].rearrange("p (h n) -> p h n", h=2)[:, :, ql:512],
                            AF.Exp, scale=SCALE)
                        if p >= 0:
                            nc.vector.tensor_mul(
                                pt[:, :, ql:ql + 128], pt[:, :, ql:ql + 128],
                                tri_sb[:, :].rearrange("p x -> p () x")
                                .broadcast_to((128, 2, 128)))
                        if debug and qb == 1 and c == 2:
                            nc.sync.dma_start(
                                dPT[:, :],
                                pt[:, :, :].rearrange("p h n -> p (h n)"))
                        pts.append((c, ql, pt))
                    stage.append(pts)
                    if len(stage) == 2:
                        _emit_y(nc, v_sb, py, stage.pop(0), ntc)
                while stage:
                    _emit_y(nc, v_sb, py, stage.pop(0), ntc)

                # ---- denominators -> 1/den replicated -> yT ----
                # dens ride in py[h] rows 64:96 (replicated); stage rows
                # {0,32} of dtot (32-aligned partition bases for DVE), one
                # [33,512] reciprocal (rows 1:31 stay 1.0), then one K=33
                # indicator-weight matmul replicates 1/den to 128 rows.
                for h in range(2):
                    hr = slice(32 * h, 32 * h + 1)
                    nc.vector.tensor_copy(dtot[hr, :], py[h][64:65, :])
                nc.vector.reciprocal(rden[0:33, :], dtot[0:33, :])
                nc.vector.tensor_copy(rdenb[0:33, :], rden[0:33, :])
                rp = pdn.tile([128, 512], f32, tag="dn", name=f"rp{qb}")
                nc.tensor.matmul(rp[:, :], sel_sb[0:33, :], rdenb[0:33, :],
                                 start=True, stop=True)
                rrs = tmp.tile([128, 512], bf16, tag="rrs", name=f"rrs{qb}")
                nc.vector.tensor_copy(rrs[:, :], rp[:, :])
                for h in range(2):
                    nc.vector.tensor_mul(yT[64 * h:64 * h + 64, q0:q0 + 512],
                                         py[h][0:64, :],
                                         rrs[64 * h:64 * h + 64, :])
                if debug:
                    dnb = tmp.tile([64, 512], bf16, tag="rrs", name=f"dnb{qb}")
                    nc.vector.tensor_copy(dnb[0:64, :], dtot[0:64, :])
                    nc.sync.dma_start(dDEN[:, q0:q0 + 512], dnb[0:64, :])

                # ---- phase 5: c_proj for the 4 finished t-tiles ----
                for tt in range(4 * qb, 4 * qb + 4):
                    ts2 = slice(128 * tt, 128 * tt + 128)
                    for half in range(2):
                        po = ppo.tile([128, 512], f32, tag="po",
                                      name=f"po{tt}_{half}")
                        nc.tensor.matmul(po[:, :], yT[:, ts2],
                                         wc_sb[:, 512 * half:512 * half + 512],
                                         start=True, stop=True)
                        ob = obpool.tile([128, 512], bf16, tag="ob",
                                         name=f"ob{tt}_{half}")
                        nc.vector.tensor_copy(ob[:, :], po[:, :])
                        nc.sync.dma_start(outp[ts2, 512 * half:512 * half + 512],
                                          ob[:, :])
            if debug:
                nc.sync.dma_start(dQT[:, :], QT[:, :])
                nc.sync.dma_start(dKT[:, :], KT[:, :])
                nc.sync.dma_start(
                    dV[:, :], v_sb[:, :, :].rearrange("p tt d -> p (tt d)"))
                nc.sync.dma_start(dYT[:, :], yT[:, :])
    _cap_matmul_waits(nc)
    return nc


def _emit_y(nc, v_sb, py, pts, ntc):
    """PV accumulation with lhsT = [V_h(64) | ones(32)]: M=96, the ones
    columns accumulate the softmax denominator for free (rows 64:96)."""
    for (c, ql, pt) in pts:
        for h in range(2):
            nc.tensor.matmul(py[h][0:96, ql:512],
                             v_sb[:, c, 96 * h:96 * h + 96],
                             pt[:, h, ql:512],
                             start=(c == 0), stop=(c == ntc - 1))


def _cap_matmul_waits(nc, limit=1):
    """walrus supports few (often one) sync-wait slots per lowered
    instruction; move excess waits onto same-engine nops inserted just
    before, so the sequencer blocks identically but each instruction
    carries at most `limit` waits."""
    import bass_rust
    from concourse import mybir

    eng = {
        mybir.EngineType.PE: nc.tensor,
        mybir.EngineType.DVE: nc.vector,
        mybir.EngineType.Activation: nc.scalar,
        mybir.EngineType.Pool: nc.gpsimd,
        mybir.EngineType.SP: nc.sync,
    }

    def make_nop(e):
        eng[e].nop()
        fn = nc.m.functions[0]
        for obb in fn.blocks:
            if (obb.instructions
                    and type(obb.instructions[-1]).__name__ == 'InstNoOp'):
                return obb.instructions.pop()
        raise AssertionError('nop not found')

    fn = nc.m.functions[0]
    for bb in fn.blocks:
        il = bb.instructions
        i = 0
        while i < len(il):
            inst = il[i]
            si = inst.sync_info
            if (si and si.on_wait and len(si.on_wait) > limit
                    and inst.engine in eng
                    and type(inst).__name__ != 'InstNoOp'):
                waits = list(si.on_wait)
                keep, excess = waits[-limit:], waits[:-limit]
                for w in excess:
                    nop = make_nop(inst.engine)
                    nop.sync_info = bass_rust.SyncInfo(on_wait=[w],
                                                       on_update=[])
                    il.insert(i, nop)
                    i += 1
                inst.sync_info = bass_rust.SyncInfo(
                    on_wait=keep, on_update=list(si.on_update))
            i += 1


def _host_prep(x, ve, qkv_w, lambdas, c_proj_w, T):
    import ml_dtypes
    bf = ml_dtypes.bfloat16
    xT = np.ascontiguousarray(x.reshape(T, DIM).T.astype(bf))
    af = (1.0 / 1024.0) ** np.linspace(0.0, 1.0, HD // 4, dtype=np.float32)
    theta = np.arange(T, dtype=np.float32)[:, None] * af[None, :]
    cos = np.ascontiguousarray(np.cos(theta).astype(bf))
    sin = np.ascontiguousarray(np.sin(theta).astype(bf))
    tri = np.ascontiguousarray(np.triu(np.ones((128, 128), np.float32)).astype(bf))
    iden = np.ascontiguousarray(np.eye(128, dtype=np.float32).astype(bf))
    lam = np.asarray(lambdas, np.float32)
    vef = ve.reshape(T, DIM)
    in_maps = []
    for c in range(NC_CORES):
        sl = slice(128 * c, 128 * c + 128)
        wq = qkv_w[0][sl]
        wk = qkv_w[1][sl]
        wv = qkv_w[2][sl] * lam[0]
        wTl = np.ascontiguousarray(np.concatenate([wq, wk, wv], 0).T.astype(bf))
        ve_l = np.ones((T, 192), np.float32)
        ve_l[:, 0:64] = vef[:, sl.start:sl.start + 64] * lam[1]
        ve_l[:, 96:160] = vef[:, sl.start + 64:sl.stop] * lam[1]
        ve_l = np.ascontiguousarray(ve_l.astype(bf))
        wcTl = np.ascontiguousarray(c_proj_w[:, sl].T.astype(bf))
        selm = np.zeros((64, 128), np.float32)
        selm[0, 0:64] = 1.0
        selm[32, 64:128] = 1.0
        in_maps.append(dict(xT=xT, wT=wTl, ve=ve_l, cosn=cos, sinn=sin,
                            tri=tri, iden=iden, wcT=wcTl,
                            sel=np.ascontiguousarray(selm.astype(bf))))
    return in_maps


LAST_RESULTS = None


def kernel(x, ve, qkv_w, lambdas, c_proj_w):
    import sys
    if '/opt/trn_rl_repo' not in sys.path:
        sys.path.insert(0, '/opt/trn_rl_repo')
    from concourse.bass_utils import run_bass_kernel_spmd

    x = np.asarray(x, np.float32)
    T = x.shape[1]
    in_maps = _host_prep(np.asarray(x, np.float32), np.asarray(ve, np.float32),
                         np.asarray(qkv_w, np.float32),
                         np.asarray(lambdas, np.float32),
                         np.asarray(c_proj_w, np.float32), T)
    nc = _build(T)
    res = run_bass_kernel_spmd(nc, in_maps, core_ids=list(range(NC_CORES)))
    global LAST_RESULTS
    LAST_RESULTS = res
    out = np.zeros((T, DIM), np.float32)
    for rmap in res.results:
        out += rmap["outp"].astype(np.float32)
    return out.reshape(1, T, DIM)
